# revision 13
# baseline (speedup 1.0000x reference)
"""GNN message-passing kernel for Trainium2 (8 NeuronCores).

Reference computation:
    out[b,i,f] = X[b,0,i,i,f] + sum_{k=1..3} sum_j A[b,i,j] * X[b,k,i,j,f]

Sharding: 8 cores = (batch b in 0..3) x (i-half h in 0..1); each core owns
a (b, 128-row i-slab). Hop 0 only contributes its diagonal, so only
X[b,1:4] (3/4 of X) plus the hop-0 diagonal rows are ever sent to the
device: ~25 MB per core.

Per-core device kernel (v2 — DVE formulation, no transpose):
  - X slabs are DMA'd in their NATURAL layout: partition = i (128 rows),
    free = (j, f) flattened. Each partition's data is one fully
    contiguous 16 KB run per (hop, j-chunk) -> near-peak HBM bandwidth.
  - The 3-hop sum is folded into the load itself with SWDGE
    accumulate-DMA (CCE inline add): hops 2,3 accumulate onto hop 1's
    tile. SBUF then holds Xs = sum_k X[k] directly.
  - out[i,f] = sum_j A[i,j] * Xs[i,j,f] is computed on the VectorEngine:
    a broadcast-AP multiply (A[i,j] broadcast over f via a 0-step AP
    dim) followed by tensor_reduce over j, per j-chunk, plus the hop-0
    diagonal added at the end.
"""

import sys

if "/opt/trn_rl_repo" not in sys.path:
    sys.path.insert(0, "/opt/trn_rl_repo")

import numpy as np

import concourse.bacc as bacc
import concourse.bass as bass
import concourse.mybir as mybir
from concourse.bass_utils import run_bass_kernel_spmd
from concourse.tile import TileContext

BATCH, KP1, N, F = 4, 4, 256, 64
NH = N // 2          # 128 rows of output per core (partition dim)
CH = 4               # j-chunks
CJ = N // CH         # j per chunk
FP32 = mybir.dt.float32
USE_ACCUM_DMA = False

_CACHE = {}


def _build_nc():
    if "nc" in _CACHE:
        return _CACHE["nc"]
    nc = bacc.Bacc("TRN2", target_bir_lowering=False, debug=False, num_devices=8)
    xk = nc.dram_tensor("xk", [3, NH, N, F], FP32, kind="ExternalInput").ap()
    a = nc.dram_tensor("a", [NH, N], FP32, kind="ExternalInput").ap()
    d = nc.dram_tensor("d", [NH, F], FP32, kind="ExternalInput").ap()
    out = nc.dram_tensor("out", [NH, F], FP32, kind="ExternalOutput").ap()

    with TileContext(nc) as tc:
        with (
            tc.tile_pool(name="const", bufs=1) as cpool,
            tc.tile_pool(name="xs", bufs=3) as xpool,
            tc.tile_pool(name="pr", bufs=2) as prpool,
            tc.tile_pool(name="sm", bufs=2) as smpool,
            tc.tile_pool(name="ac", bufs=1) as acpool,
        ):
            a_sb = cpool.tile([128, N], FP32)
            nc.scalar.dma_start(out=a_sb[:, :], in_=a[:, :])
            d_sb = cpool.tile([128, F], FP32)
            nc.scalar.dma_start(out=d_sb[:, :], in_=d[:, :])

            acc = acpool.tile([128, F], FP32)
            # per-chunk partials land side by side; combined once at the end
            parts = acpool.tile([128, CH * F], FP32)

            for c in range(CH):
                xs = xpool.tile([128, CJ * F], FP32, name="xs", tag="xs")
                xs_step = xs.ap[0][0]
                if USE_ACCUM_DMA:
                    # CCE (the inline DMA adder) handles at most 2048
                    # elements per descriptor; split each partition's
                    # 4096-elem run into two 2048-elem runs.
                    nrun = (CJ * F) // 2048
                    for k in range(3):
                        src = bass.AP(
                            xk.tensor,
                            k * NH * N * F + c * CJ * F,
                            [[N * F, 128], [2048, nrun], [1, 2048]],
                        )
                        dst = bass.AP(
                            xs.tensor,
                            0,
                            [[xs_step, 128], [2048, nrun], [1, 2048]],
                        )
                        nc.gpsimd.dma_start(
                            out=dst,
                            in_=src,
                            accum_op=(
                                mybir.AluOpType.add
                                if k > 0
                                else mybir.AluOpType.bypass
                            ),
                        )
                else:
                    xts = [xs]
                    for k in (1, 2):
                        xt = xpool.tile(
                            [128, CJ * F], FP32, name=f"xt{k}", tag=f"xt{k}"
                        )
                        xts.append(xt)
                    for k in range(3):
                        src = bass.AP(
                            xk.tensor,
                            k * NH * N * F + c * CJ * F,
                            [[N * F, 128], [1, CJ * F]],
                        )
                        eng = nc.sync if k % 2 == 0 else nc.scalar
                        eng.dma_start(out=xts[k][:, :], in_=src)
                    # g01 = x0+x1 on GpSimd (starts after 2nd DMA, runs
                    # parallel with DVE); xs = g01+x2 on DVE
                    nc.gpsimd.tensor_add(xts[0][:, :], xts[0][:, :], xts[1][:, :])
                    nc.vector.tensor_add(xs[:, :], xts[0][:, :], xts[2][:, :])

                # prod[i, j*F+f] = xs[i, j*F+f] * a_sb[i, c*CJ+j]
                prod = prpool.tile([128, CJ * F], FP32, name="prod", tag="prod")
                pr_step = prod.ap[0][0]
                a_step = a_sb.ap[0][0]
                in0 = bass.AP(xs.tensor, 0, [[xs_step, 128], [F, CJ], [1, F]])
                in1 = bass.AP(
                    a_sb.tensor, c * CJ, [[a_step, 128], [1, CJ], [0, F]]
                )
                po = bass.AP(prod.tensor, 0, [[pr_step, 128], [F, CJ], [1, F]])
                nc.vector.tensor_mul(po, in0, in1)

                # parts[:, c*F:(c+1)*F] = sum_j prod[i, j*F+f]
                pin = bass.AP(prod.tensor, 0, [[pr_step, 128], [1, F], [F, CJ]])
                nc.vector.reduce_sum(
                    parts[:, c * F : (c + 1) * F], pin, axis=mybir.AxisListType.X
                )

            # acc = d + sum_c parts[c]  (one strided reduce + one add)
            psum_all = smpool.tile([128, F], FP32)
            a2_step = parts.ap[0][0]
            fin = bass.AP(parts.tensor, 0, [[a2_step, 128], [1, F], [F, CH]])
            nc.vector.reduce_sum(psum_all[:, :], fin, axis=mybir.AxisListType.X)
            nc.vector.tensor_add(acc[:, :], d_sb[:, :], psum_all[:, :])

            nc.sync.dma_start(out=out[:, :], in_=acc[:, :])

    nc.compile()
    _CACHE["nc"] = nc
    return nc


def _make_in_maps(A, X):
    idx = np.arange(NH)
    in_maps = []
    for c in range(8):
        b, h = c // 2, c % 2
        lo = h * NH
        xk = np.ascontiguousarray(X[b, 1:4, lo : lo + NH])
        av = np.ascontiguousarray(A[b, lo : lo + NH, :])
        dv = np.ascontiguousarray(X[b, 0, lo + idx, lo + idx, :])
        in_maps.append({"xk": xk, "a": av, "d": dv})
    return in_maps


def run(A, X, trace=False, **kw):
    nc = _build_nc()
    in_maps = _make_in_maps(A, X)
    res = run_bass_kernel_spmd(
        nc, in_maps, core_ids=list(range(8)), trace=trace, **kw
    )
    out = np.empty((BATCH, N, F), dtype=np.float32)
    for c in range(8):
        b, h = c // 2, c % 2
        out[b, h * NH : (h + 1) * NH] = res.results[c]["out"]
    return out, res


def kernel(A, X):
    A = np.asarray(A, dtype=np.float32)
    X = np.asarray(X, dtype=np.float32)
    out, _ = run(A, X, trace=False)
    return out


# revision 16
# speedup vs baseline: 1.0203x; 1.0203x over previous
"""GNN message-passing kernel for Trainium2 (8 NeuronCores).

Reference computation:
    out[b,i,f] = X[b,0,i,i,f] + sum_{k=1..3} sum_j A[b,i,j] * X[b,k,i,j,f]

Sharding: 8 cores = (batch b in 0..3) x (i-half h in 0..1); each core owns
a (b, 128-row i-slab). Hop 0 only contributes its diagonal, so only
X[b,1:4] (3/4 of X) plus the hop-0 diagonal rows are ever sent to the
device: ~25 MB per core.

Per-core device kernel (v2 — DVE formulation, no transpose):
  - X slabs are DMA'd in their NATURAL layout: partition = i (128 rows),
    free = (j, f) flattened. Each partition's data is one fully
    contiguous 16 KB run per (hop, j-chunk) -> near-peak HBM bandwidth.
  - The 3-hop sum is folded into the load itself with SWDGE
    accumulate-DMA (CCE inline add): hops 2,3 accumulate onto hop 1's
    tile. SBUF then holds Xs = sum_k X[k] directly.
  - out[i,f] = sum_j A[i,j] * Xs[i,j,f] is computed on the VectorEngine:
    a broadcast-AP multiply (A[i,j] broadcast over f via a 0-step AP
    dim) followed by tensor_reduce over j, per j-chunk, plus the hop-0
    diagonal added at the end.
"""

import sys

if "/opt/trn_rl_repo" not in sys.path:
    sys.path.insert(0, "/opt/trn_rl_repo")

import numpy as np

import concourse.bacc as bacc
import concourse.bass as bass
import concourse.mybir as mybir
from concourse.bass_utils import run_bass_kernel_spmd
from concourse.tile import TileContext

BATCH, KP1, N, F = 4, 4, 256, 64
NH = N // 2          # 128 rows of output per core (partition dim)
CH = 8               # j-chunks
CJ = N // CH         # j per chunk
FP32 = mybir.dt.float32

_CACHE = {}


def _build_nc():
    if "nc" in _CACHE:
        return _CACHE["nc"]
    nc = bacc.Bacc("TRN2", target_bir_lowering=False, debug=False, num_devices=8)
    xk = nc.dram_tensor("xk", [3, NH, N, F], FP32, kind="ExternalInput").ap()
    a = nc.dram_tensor("a", [NH, N], FP32, kind="ExternalInput").ap()
    d = nc.dram_tensor("d", [NH, F], FP32, kind="ExternalInput").ap()
    eye = nc.dram_tensor("eye", [128, 128], FP32, kind="ExternalInput").ap()
    out = nc.dram_tensor("out", [NH, F], FP32, kind="ExternalOutput").ap()

    NS = (CJ * F) // 512  # 512-col matmul slices per chunk

    with TileContext(nc) as tc:
        with (
            tc.tile_pool(name="const", bufs=1) as cpool,
            tc.tile_pool(name="xs", bufs=3) as xpool,
            tc.tile_pool(name="pr", bufs=3) as prpool,
            tc.tile_pool(name="sm", bufs=2) as smpool,
            tc.tile_pool(name="ac", bufs=1) as acpool,
            tc.tile_pool(name="ps", bufs=2, space="PSUM") as pspool,
        ):
            a_sb = cpool.tile([128, N], FP32)
            nc.scalar.dma_start(out=a_sb[:, :], in_=a[:, :])
            d_sb = cpool.tile([128, F], FP32)
            nc.scalar.dma_start(out=d_sb[:, :], in_=d[:, :])
            eye_sb = cpool.tile([128, 128], FP32)
            nc.scalar.dma_start(out=eye_sb[:, :], in_=eye[:, :])

            acc = acpool.tile([128, F], FP32)
            # per-chunk partials land side by side; combined once at the end
            parts = acpool.tile([128, CH * F], FP32)

            for c in range(CH):
                xts = []
                for k in range(3):
                    xt = xpool.tile(
                        [128, CJ * F], FP32, name=f"xt{k}", tag=f"xt{k}"
                    )
                    src = bass.AP(
                        xk.tensor,
                        k * NH * N * F + c * CJ * F,
                        [[N * F, 128], [1, CJ * F]],
                    )
                    eng = nc.sync if k % 2 == 0 else nc.scalar
                    eng.dma_start(out=xt[:, :], in_=src)
                    xts.append(xt)

                # hop-sum on the TensorEngine: identity-stationary matmuls
                # accumulate x1+x2+x3 into PSUM (PE has its own SBUF read
                # ports -> no DVE contention)
                ps = pspool.tile([128, CJ * F], FP32, name="ps", tag="ps")
                for s in range(NS):
                    sl = slice(s * 512, (s + 1) * 512)
                    for k in range(3):
                        nc.tensor.matmul(
                            ps[:, sl],
                            eye_sb[:, :],
                            xts[k][:, sl],
                            start=(k == 0),
                            stop=(k == 2),
                        )

                # prod[i, j*F+f] = ps[i, j*F+f] * a_sb[i, c*CJ+j]
                prod = prpool.tile([128, CJ * F], FP32, name="prod", tag="prod")
                pr_step = prod.ap[0][0]
                ps_step = ps.ap[0][0]
                a_step = a_sb.ap[0][0]
                in0 = bass.AP(ps.tensor, 0, [[ps_step, 128], [F, CJ], [1, F]])
                in1 = bass.AP(
                    a_sb.tensor, c * CJ, [[a_step, 128], [1, CJ], [0, F]]
                )
                po = bass.AP(prod.tensor, 0, [[pr_step, 128], [F, CJ], [1, F]])
                nc.vector.tensor_mul(po, in0, in1)

                # parts[:, c*F:(c+1)*F] = sum_j prod[i, j*F+f]
                pin = bass.AP(prod.tensor, 0, [[pr_step, 128], [1, F], [F, CJ]])
                nc.vector.reduce_sum(
                    parts[:, c * F : (c + 1) * F], pin, axis=mybir.AxisListType.X
                )

            # acc = d + sum_c parts[c]  (one strided reduce + one add)
            psum_all = smpool.tile([128, F], FP32)
            a2_step = parts.ap[0][0]
            fin = bass.AP(parts.tensor, 0, [[a2_step, 128], [1, F], [F, CH]])
            nc.vector.reduce_sum(psum_all[:, :], fin, axis=mybir.AxisListType.X)
            nc.vector.tensor_add(acc[:, :], d_sb[:, :], psum_all[:, :])

            nc.sync.dma_start(out=out[:, :], in_=acc[:, :])

    nc.compile()
    _CACHE["nc"] = nc
    return nc


def _make_in_maps(A, X):
    idx = np.arange(NH)
    in_maps = []
    for c in range(8):
        b, h = c // 2, c % 2
        lo = h * NH
        xk = np.ascontiguousarray(X[b, 1:4, lo : lo + NH])
        av = np.ascontiguousarray(A[b, lo : lo + NH, :])
        dv = np.ascontiguousarray(X[b, 0, lo + idx, lo + idx, :])
        in_maps.append(
            {"xk": xk, "a": av, "d": dv, "eye": np.eye(128, dtype=np.float32)}
        )
    return in_maps


def run(A, X, trace=False, **kw):
    nc = _build_nc()
    in_maps = _make_in_maps(A, X)
    res = run_bass_kernel_spmd(
        nc, in_maps, core_ids=list(range(8)), trace=trace, **kw
    )
    out = np.empty((BATCH, N, F), dtype=np.float32)
    for c in range(8):
        b, h = c // 2, c % 2
        out[b, h * NH : (h + 1) * NH] = res.results[c]["out"]
    return out, res


def kernel(A, X):
    A = np.asarray(A, dtype=np.float32)
    X = np.asarray(X, dtype=np.float32)
    out, _ = run(A, X, trace=False)
    return out


# revision 17
# speedup vs baseline: 1.1032x; 1.0812x over previous
"""GNN message-passing kernel for Trainium2 (8 NeuronCores).

Reference computation:
    out[b,i,f] = X[b,0,i,i,f] + sum_{k=1..3} sum_j A[b,i,j] * X[b,k,i,j,f]

Sharding: 8 cores = (batch b in 0..3) x (i-half h in 0..1); each core owns
a (b, 128-row i-slab). Hop 0 only contributes its diagonal, so only
X[b,1:4] (3/4 of X) plus the hop-0 diagonal rows are ever sent to the
device: ~25 MB per core.

Per-core device kernel (v2 — DVE formulation, no transpose):
  - X slabs are DMA'd in their NATURAL layout: partition = i (128 rows),
    free = (j, f) flattened. Each partition's data is one fully
    contiguous 16 KB run per (hop, j-chunk) -> near-peak HBM bandwidth.
  - The 3-hop sum is folded into the load itself with SWDGE
    accumulate-DMA (CCE inline add): hops 2,3 accumulate onto hop 1's
    tile. SBUF then holds Xs = sum_k X[k] directly.
  - out[i,f] = sum_j A[i,j] * Xs[i,j,f] is computed on the VectorEngine:
    a broadcast-AP multiply (A[i,j] broadcast over f via a 0-step AP
    dim) followed by tensor_reduce over j, per j-chunk, plus the hop-0
    diagonal added at the end.
"""

import sys

if "/opt/trn_rl_repo" not in sys.path:
    sys.path.insert(0, "/opt/trn_rl_repo")

import numpy as np

import concourse.bacc as bacc
import concourse.bass as bass
import concourse.mybir as mybir
from concourse.bass_utils import run_bass_kernel_spmd
from concourse.tile import TileContext

BATCH, KP1, N, F = 4, 4, 256, 64
NH = N // 2          # 128 rows of output per core (partition dim)
CH = 8               # j-chunks
CJ = N // CH         # j per chunk
FP32 = mybir.dt.float32

_CACHE = {}


def _build_nc():
    if "nc" in _CACHE:
        return _CACHE["nc"]
    nc = bacc.Bacc("TRN2", target_bir_lowering=False, debug=False, num_devices=8)
    xk = nc.dram_tensor("xk", [3, NH, N, F], FP32, kind="ExternalInput").ap()
    a = nc.dram_tensor("a", [NH, N], FP32, kind="ExternalInput").ap()
    d = nc.dram_tensor("d", [NH, F], FP32, kind="ExternalInput").ap()
    eye = nc.dram_tensor("eye", [128, 128], FP32, kind="ExternalInput").ap()
    out = nc.dram_tensor("out", [NH, F], FP32, kind="ExternalOutput").ap()

    NS = (CJ * F) // 512  # 512-col matmul slices per chunk

    with TileContext(nc) as tc:
        with (
            tc.tile_pool(name="const", bufs=1) as cpool,
            tc.tile_pool(name="xs", bufs=3) as xpool,
            tc.tile_pool(name="pr", bufs=3) as prpool,
            tc.tile_pool(name="sm", bufs=2) as smpool,
            tc.tile_pool(name="ac", bufs=1) as acpool,
            tc.tile_pool(name="ps", bufs=2, space="PSUM") as pspool,
        ):
            a_sb = cpool.tile([128, N], FP32)
            nc.scalar.dma_start(out=a_sb[:, :], in_=a[:, :])
            d_sb = cpool.tile([128, F], FP32)
            nc.scalar.dma_start(out=d_sb[:, :], in_=d[:, :])
            eye_sb = cpool.tile([128, 128], FP32)
            nc.scalar.dma_start(out=eye_sb[:, :], in_=eye[:, :])

            acc = acpool.tile([128, F], FP32)
            # per-chunk partials land side by side; combined once at the end
            parts = acpool.tile([128, CH * F], FP32)

            for c in range(CH):
                use_pe = c < CH // 2
                xts = []
                for k in range(3):
                    xt = xpool.tile(
                        [128, CJ * F], FP32, name=f"xt{k}", tag=f"xt{k}"
                    )
                    src = bass.AP(
                        xk.tensor,
                        k * NH * N * F + c * CJ * F,
                        [[N * F, 128], [1, CJ * F]],
                    )
                    eng = nc.sync if k % 2 == 0 else nc.scalar
                    eng.dma_start(out=xt[:, :], in_=src)
                    xts.append(xt)

                if use_pe:
                    # hop-sum on the TensorEngine: identity-stationary
                    # matmuls accumulate x1+x2+x3 into PSUM (PE has its own
                    # SBUF read ports -> no DVE contention). fp32 matmul is
                    # dual-pass (~590ns/512 cols), so PE only takes half
                    # the chunks; grouped first so HAM stays warm.
                    ps = pspool.tile([128, CJ * F], FP32, name="ps", tag="ps")
                    for s in range(NS):
                        sl = slice(s * 512, (s + 1) * 512)
                        for k in range(3):
                            nc.tensor.matmul(
                                ps[:, sl],
                                eye_sb[:, :],
                                xts[k][:, sl],
                                start=(k == 0),
                                stop=(k == 2),
                            )
                    xsum, xs_step = ps, ps.ap[0][0]
                else:
                    # hop-sum on DVE for the back half of the chunks
                    nc.vector.tensor_add(xts[1][:, :], xts[1][:, :], xts[2][:, :])
                    nc.vector.tensor_add(xts[0][:, :], xts[0][:, :], xts[1][:, :])
                    xsum, xs_step = xts[0], xts[0].ap[0][0]

                # prod[i, j*F+f] = xsum[i, j*F+f] * a_sb[i, c*CJ+j]
                prod = prpool.tile([128, CJ * F], FP32, name="prod", tag="prod")
                pr_step = prod.ap[0][0]
                a_step = a_sb.ap[0][0]
                in0 = bass.AP(xsum.tensor, 0, [[xs_step, 128], [F, CJ], [1, F]])
                in1 = bass.AP(
                    a_sb.tensor, c * CJ, [[a_step, 128], [1, CJ], [0, F]]
                )
                po = bass.AP(prod.tensor, 0, [[pr_step, 128], [F, CJ], [1, F]])
                nc.vector.tensor_mul(po, in0, in1)

                # parts[:, c*F:(c+1)*F] = sum_j prod[i, j*F+f]
                pin = bass.AP(prod.tensor, 0, [[pr_step, 128], [1, F], [F, CJ]])
                nc.vector.reduce_sum(
                    parts[:, c * F : (c + 1) * F], pin, axis=mybir.AxisListType.X
                )

            # acc = d + sum_c parts[c]  (one strided reduce + one add)
            psum_all = smpool.tile([128, F], FP32)
            a2_step = parts.ap[0][0]
            fin = bass.AP(parts.tensor, 0, [[a2_step, 128], [1, F], [F, CH]])
            nc.vector.reduce_sum(psum_all[:, :], fin, axis=mybir.AxisListType.X)
            nc.vector.tensor_add(acc[:, :], d_sb[:, :], psum_all[:, :])

            nc.sync.dma_start(out=out[:, :], in_=acc[:, :])

    nc.compile()
    _CACHE["nc"] = nc
    return nc


def _make_in_maps(A, X):
    idx = np.arange(NH)
    in_maps = []
    for c in range(8):
        b, h = c // 2, c % 2
        lo = h * NH
        xk = np.ascontiguousarray(X[b, 1:4, lo : lo + NH])
        av = np.ascontiguousarray(A[b, lo : lo + NH, :])
        dv = np.ascontiguousarray(X[b, 0, lo + idx, lo + idx, :])
        in_maps.append(
            {"xk": xk, "a": av, "d": dv, "eye": np.eye(128, dtype=np.float32)}
        )
    return in_maps


def run(A, X, trace=False, **kw):
    nc = _build_nc()
    in_maps = _make_in_maps(A, X)
    res = run_bass_kernel_spmd(
        nc, in_maps, core_ids=list(range(8)), trace=trace, **kw
    )
    out = np.empty((BATCH, N, F), dtype=np.float32)
    for c in range(8):
        b, h = c // 2, c % 2
        out[b, h * NH : (h + 1) * NH] = res.results[c]["out"]
    return out, res


def kernel(A, X):
    A = np.asarray(A, dtype=np.float32)
    X = np.asarray(X, dtype=np.float32)
    out, _ = run(A, X, trace=False)
    return out


# revision 18
# speedup vs baseline: 1.1251x; 1.0199x over previous
"""GNN message-passing kernel for Trainium2 (8 NeuronCores).

Reference computation:
    out[b,i,f] = X[b,0,i,i,f] + sum_{k=1..3} sum_j A[b,i,j] * X[b,k,i,j,f]

Sharding: 8 cores = (batch b in 0..3) x (i-half h in 0..1); each core owns
a (b, 128-row i-slab) of the output. Hop 0 only contributes its diagonal,
so only X[b,1:4] (3/4 of X) plus the hop-0 diagonal rows are ever sent to
the device: ~25 MB per core.

Per-core device kernel (VectorEngine formulation, no transpose):
  - X slabs are DMA'd in their NATURAL layout: partition = i (128 rows),
    free = (j, f) flattened, in 4 j-chunks of 2 MB per hop. Each
    partition's data is one fully contiguous 16 KB run -> near-peak HBM
    bandwidth (~414 GB/s measured vs ~193 GB/s for a transposed layout).
  - out[i,f] = sum_j A[i,j] * (sum_k X[k])[i,j,f] is computed on the
    VectorEngine: two adds for the hop sum, a broadcast-AP multiply
    (A[i,j] broadcast over f via a 0-step AP dim), and a strided
    tensor_reduce over j per chunk; the hop-0 diagonal is added into the
    running accumulator.

Measured on 8 axon-tunneled trn2 cores: ~112 us HW exec, rel err ~2e-7.
(DMA ~61 us at ~414 GB/s and DVE ~84 us, overlapped. Variants tried and
rejected: j-on-partition matmul formulation (162 us, 256B DMA descriptors
dominate), SWDGE accumulate-DMA for the hop sum (device crash), GpSimd
assist (SBUF port contention slows DVE), TensorEngine identity-matmul
hop-sum (fp32 dual-pass makes PE the bottleneck, 114-123 us).)
"""

import sys

if "/opt/trn_rl_repo" not in sys.path:
    sys.path.insert(0, "/opt/trn_rl_repo")

import numpy as np

import concourse.bacc as bacc
import concourse.bass as bass
import concourse.mybir as mybir
from concourse.bass_utils import run_bass_kernel_spmd
from concourse.tile import TileContext

BATCH, KP1, N, F = 4, 4, 256, 64
NH = N // 2          # 128 rows of output per core (partition dim)
CH = 4               # j-chunks
CJ = N // CH         # j per chunk
FP32 = mybir.dt.float32

_CACHE = {}


def _build_nc():
    if "nc" in _CACHE:
        return _CACHE["nc"]
    nc = bacc.Bacc("TRN2", target_bir_lowering=False, debug=False, num_devices=8)
    xk = nc.dram_tensor("xk", [3, NH, N, F], FP32, kind="ExternalInput").ap()
    a = nc.dram_tensor("a", [NH, N], FP32, kind="ExternalInput").ap()
    d = nc.dram_tensor("d", [NH, F], FP32, kind="ExternalInput").ap()
    out = nc.dram_tensor("out", [NH, F], FP32, kind="ExternalOutput").ap()

    with TileContext(nc) as tc:
        with (
            tc.tile_pool(name="const", bufs=1) as cpool,
            tc.tile_pool(name="xs", bufs=3) as xpool,
            tc.tile_pool(name="pr", bufs=2) as prpool,
            tc.tile_pool(name="sm", bufs=2) as smpool,
            tc.tile_pool(name="ac", bufs=1) as acpool,
        ):
            a_sb = cpool.tile([128, N], FP32)
            nc.sync.dma_start(out=a_sb[:, :], in_=a[:, :])
            d_sb = cpool.tile([128, F], FP32)
            nc.sync.dma_start(out=d_sb[:, :], in_=d[:, :])

            acc = acpool.tile([128, F], FP32)

            for c in range(CH):
                xts = []
                for k in range(3):
                    xt = xpool.tile(
                        [128, CJ * F], FP32, name=f"xt{k}", tag=f"xt{k}"
                    )
                    src = bass.AP(
                        xk.tensor,
                        k * NH * N * F + c * CJ * F,
                        [[N * F, 128], [1, CJ * F]],
                    )
                    nc.sync.dma_start(out=xt[:, :], in_=src)
                    xts.append(xt)
                # hop sum on DVE (in place)
                nc.vector.tensor_add(xts[1][:, :], xts[1][:, :], xts[2][:, :])
                nc.vector.tensor_add(xts[0][:, :], xts[0][:, :], xts[1][:, :])
                xs = xts[0]
                xs_step = xs.ap[0][0]

                # prod[i, j*F+f] = xs[i, j*F+f] * a_sb[i, c*CJ+j]
                prod = prpool.tile([128, CJ * F], FP32, name="prod", tag="prod")
                pr_step = prod.ap[0][0]
                a_step = a_sb.ap[0][0]
                in0 = bass.AP(xs.tensor, 0, [[xs_step, 128], [F, CJ], [1, F]])
                in1 = bass.AP(
                    a_sb.tensor, c * CJ, [[a_step, 128], [1, CJ], [0, F]]
                )
                po = bass.AP(prod.tensor, 0, [[pr_step, 128], [F, CJ], [1, F]])
                nc.vector.tensor_mul(po, in0, in1)

                # partial[i, f] = sum_j prod[i, j*F+f]  (reduce innermost=j)
                partial = smpool.tile([128, F], FP32, name="partial", tag="partial")
                pin = bass.AP(prod.tensor, 0, [[pr_step, 128], [1, F], [F, CJ]])
                nc.vector.reduce_sum(
                    partial[:, :], pin, axis=mybir.AxisListType.X
                )

                if c == 0:
                    nc.vector.tensor_add(acc[:, :], d_sb[:, :], partial[:, :])
                else:
                    nc.vector.tensor_add(acc[:, :], acc[:, :], partial[:, :])

            nc.sync.dma_start(out=out[:, :], in_=acc[:, :])

    nc.compile()
    _CACHE["nc"] = nc
    return nc


def _make_in_maps(A, X):
    idx = np.arange(NH)
    in_maps = []
    for c in range(8):
        b, h = c // 2, c % 2
        lo = h * NH
        xk = np.ascontiguousarray(X[b, 1:4, lo : lo + NH])
        av = np.ascontiguousarray(A[b, lo : lo + NH, :])
        dv = np.ascontiguousarray(X[b, 0, lo + idx, lo + idx, :])
        in_maps.append({"xk": xk, "a": av, "d": dv})
    return in_maps


def run(A, X, trace=False, **kw):
    nc = _build_nc()
    in_maps = _make_in_maps(A, X)
    res = run_bass_kernel_spmd(
        nc, in_maps, core_ids=list(range(8)), trace=trace, **kw
    )
    out = np.empty((BATCH, N, F), dtype=np.float32)
    for c in range(8):
        b, h = c // 2, c % 2
        out[b, h * NH : (h + 1) * NH] = res.results[c]["out"]
    return out, res


def kernel(A, X):
    A = np.asarray(A, dtype=np.float32)
    X = np.asarray(X, dtype=np.float32)
    out, _ = run(A, X, trace=False)
    return out


# revision 19
# speedup vs baseline: 1.1374x; 1.0109x over previous
"""GNN message-passing kernel for Trainium2 (8 NeuronCores).

Reference computation:
    out[b,i,f] = X[b,0,i,i,f] + sum_{k=1..3} sum_j A[b,i,j] * X[b,k,i,j,f]

Sharding: 8 cores = (batch b in 0..3) x (i-half h in 0..1); each core owns
a (b, 128-row i-slab) of the output. Hop 0 only contributes its diagonal,
so only X[b,1:4] (3/4 of X) plus the hop-0 diagonal rows are ever sent to
the device: ~25 MB per core.

Per-core device kernel (VectorEngine formulation, no transpose):
  - X slabs are DMA'd in their NATURAL layout: partition = i (128 rows),
    free = (j, f) flattened, in 4 j-chunks of 2 MB per hop. Each
    partition's data is one fully contiguous 16 KB run -> near-peak HBM
    bandwidth (~414 GB/s measured vs ~193 GB/s for a transposed layout).
  - out[i,f] = sum_j A[i,j] * (sum_k X[k])[i,j,f] is computed on the
    VectorEngine: two adds for the hop sum, a broadcast-AP multiply
    (A[i,j] broadcast over f via a 0-step AP dim), and a strided
    tensor_reduce over j per chunk; the hop-0 diagonal is added into the
    running accumulator.

Measured on 8 axon-tunneled trn2 cores: ~112 us HW exec, rel err ~2e-7.
(DMA ~61 us at ~414 GB/s and DVE ~84 us, overlapped. Variants tried and
rejected: j-on-partition matmul formulation (162 us, 256B DMA descriptors
dominate), SWDGE accumulate-DMA for the hop sum (device crash), GpSimd
assist (SBUF port contention slows DVE), TensorEngine identity-matmul
hop-sum (fp32 dual-pass makes PE the bottleneck, 114-123 us).)
"""

import sys

if "/opt/trn_rl_repo" not in sys.path:
    sys.path.insert(0, "/opt/trn_rl_repo")

import numpy as np

import concourse.bacc as bacc
import concourse.bass as bass
import concourse.mybir as mybir
from concourse.bass_utils import run_bass_kernel_spmd
from concourse.tile import TileContext

BATCH, KP1, N, F = 4, 4, 256, 64
NH = N // 2          # 128 rows of output per core (partition dim)
# j-chunk sizes: small chunks first so DVE starts ~7us earlier
CJS = [32, 32, 64, 64, 64]
FP32 = mybir.dt.float32

_CACHE = {}


def _build_nc():
    if "nc" in _CACHE:
        return _CACHE["nc"]
    nc = bacc.Bacc("TRN2", target_bir_lowering=False, debug=False, num_devices=8)
    xk = nc.dram_tensor("xk", [3, NH, N, F], FP32, kind="ExternalInput").ap()
    a = nc.dram_tensor("a", [NH, N], FP32, kind="ExternalInput").ap()
    d = nc.dram_tensor("d", [NH, F], FP32, kind="ExternalInput").ap()
    out = nc.dram_tensor("out", [NH, F], FP32, kind="ExternalOutput").ap()

    with TileContext(nc) as tc:
        with (
            tc.tile_pool(name="const", bufs=1) as cpool,
            tc.tile_pool(name="xs", bufs=3) as xpool,
            tc.tile_pool(name="pr", bufs=2) as prpool,
            tc.tile_pool(name="sm", bufs=2) as smpool,
            tc.tile_pool(name="ac", bufs=1) as acpool,
        ):
            a_sb = cpool.tile([128, N], FP32)
            nc.sync.dma_start(out=a_sb[:, :], in_=a[:, :])
            d_sb = cpool.tile([128, F], FP32)
            nc.sync.dma_start(out=d_sb[:, :], in_=d[:, :])

            acc = acpool.tile([128, F], FP32)

            j0 = 0
            for c, CJ in enumerate(CJS):
                xts = []
                for k in range(3):
                    xt = xpool.tile(
                        [128, CJ * F], FP32, name=f"xt{k}", tag=f"xt{k}"
                    )
                    src = bass.AP(
                        xk.tensor,
                        k * NH * N * F + j0 * F,
                        [[N * F, 128], [1, CJ * F]],
                    )
                    nc.sync.dma_start(out=xt[:, :], in_=src)
                    xts.append(xt)
                # hop sum on DVE (in place)
                nc.vector.tensor_add(xts[1][:, :], xts[1][:, :], xts[2][:, :])
                nc.vector.tensor_add(xts[0][:, :], xts[0][:, :], xts[1][:, :])
                xs = xts[0]
                xs_step = xs.ap[0][0]

                # prod[i, j*F+f] = xs[i, j*F+f] * a_sb[i, c*CJ+j]
                prod = prpool.tile([128, CJ * F], FP32, name="prod", tag="prod")
                pr_step = prod.ap[0][0]
                a_step = a_sb.ap[0][0]
                in0 = bass.AP(xs.tensor, 0, [[xs_step, 128], [F, CJ], [1, F]])
                in1 = bass.AP(
                    a_sb.tensor, j0, [[a_step, 128], [1, CJ], [0, F]]
                )
                j0 += CJ
                po = bass.AP(prod.tensor, 0, [[pr_step, 128], [F, CJ], [1, F]])
                nc.vector.tensor_mul(po, in0, in1)

                # partial[i, f] = sum_j prod[i, j*F+f]  (reduce innermost=j)
                partial = smpool.tile([128, F], FP32, name="partial", tag="partial")
                pin = bass.AP(prod.tensor, 0, [[pr_step, 128], [1, F], [F, CJ]])
                nc.vector.reduce_sum(
                    partial[:, :], pin, axis=mybir.AxisListType.X
                )

                if c == 0:
                    nc.vector.tensor_add(acc[:, :], d_sb[:, :], partial[:, :])
                else:
                    nc.vector.tensor_add(acc[:, :], acc[:, :], partial[:, :])

            nc.sync.dma_start(out=out[:, :], in_=acc[:, :])

    nc.compile()
    _CACHE["nc"] = nc
    return nc


def _make_in_maps(A, X):
    idx = np.arange(NH)
    in_maps = []
    for c in range(8):
        b, h = c // 2, c % 2
        lo = h * NH
        xk = np.ascontiguousarray(X[b, 1:4, lo : lo + NH])
        av = np.ascontiguousarray(A[b, lo : lo + NH, :])
        dv = np.ascontiguousarray(X[b, 0, lo + idx, lo + idx, :])
        in_maps.append({"xk": xk, "a": av, "d": dv})
    return in_maps


def run(A, X, trace=False, **kw):
    nc = _build_nc()
    in_maps = _make_in_maps(A, X)
    res = run_bass_kernel_spmd(
        nc, in_maps, core_ids=list(range(8)), trace=trace, **kw
    )
    out = np.empty((BATCH, N, F), dtype=np.float32)
    for c in range(8):
        b, h = c // 2, c % 2
        out[b, h * NH : (h + 1) * NH] = res.results[c]["out"]
    return out, res


def kernel(A, X):
    A = np.asarray(A, dtype=np.float32)
    X = np.asarray(X, dtype=np.float32)
    out, _ = run(A, X, trace=False)
    return out


# revision 20
# speedup vs baseline: 1.1677x; 1.0266x over previous
"""GNN message-passing kernel for Trainium2 (8 NeuronCores).

Reference computation:
    out[b,i,f] = X[b,0,i,i,f] + sum_{k=1..3} sum_j A[b,i,j] * X[b,k,i,j,f]

Sharding: 8 cores = (batch b in 0..3) x (i-half h in 0..1); each core owns
a (b, 128-row i-slab) of the output. Hop 0 only contributes its diagonal,
so only X[b,1:4] (3/4 of X) plus the hop-0 diagonal rows are ever sent to
the device: ~25 MB per core.

Per-core device kernel (VectorEngine formulation, no transpose):
  - X slabs are DMA'd in their NATURAL layout: partition = i (128 rows),
    free = (j, f) flattened, in 4 j-chunks of 2 MB per hop. Each
    partition's data is one fully contiguous 16 KB run -> near-peak HBM
    bandwidth (~414 GB/s measured vs ~193 GB/s for a transposed layout).
  - out[i,f] = sum_j A[i,j] * (sum_k X[k])[i,j,f] is computed on the
    VectorEngine: two adds for the hop sum, a broadcast-AP multiply
    (A[i,j] broadcast over f via a 0-step AP dim), and a strided
    tensor_reduce over j per chunk; the hop-0 diagonal is added into the
    running accumulator.

Measured on 8 axon-tunneled trn2 cores: ~112 us HW exec, rel err ~2e-7.
(DMA ~61 us at ~414 GB/s and DVE ~84 us, overlapped. Variants tried and
rejected: j-on-partition matmul formulation (162 us, 256B DMA descriptors
dominate), SWDGE accumulate-DMA for the hop sum (device crash), GpSimd
assist (SBUF port contention slows DVE), TensorEngine identity-matmul
hop-sum (fp32 dual-pass makes PE the bottleneck, 114-123 us).)
"""

import sys

if "/opt/trn_rl_repo" not in sys.path:
    sys.path.insert(0, "/opt/trn_rl_repo")

import numpy as np

import concourse.bacc as bacc
import concourse.bass as bass
import concourse.mybir as mybir
from concourse.bass_utils import run_bass_kernel_spmd
from concourse.tile import TileContext

BATCH, KP1, N, F = 4, 4, 256, 64
NH = N // 2          # 128 rows of output per core (partition dim)
# j-chunk sizes: small chunks first so DVE starts ~7us earlier.
# Chunks 1,2 get their hop-sum done on the TensorEngine (identity-matmul
# accumulate into PSUM) -- back-to-back so the HAM cold-start is paid once.
CJS = [32, 32, 32, 64, 64, 32]
PE_CHUNKS = {1, 2}
FP32 = mybir.dt.float32

_CACHE = {}


def _build_nc():
    if "nc" in _CACHE:
        return _CACHE["nc"]
    nc = bacc.Bacc("TRN2", target_bir_lowering=False, debug=False, num_devices=8)
    xk = nc.dram_tensor("xk", [3, NH, N, F], FP32, kind="ExternalInput").ap()
    a = nc.dram_tensor("a", [NH, N], FP32, kind="ExternalInput").ap()
    d = nc.dram_tensor("d", [NH, F], FP32, kind="ExternalInput").ap()
    eye = nc.dram_tensor("eye", [128, 128], FP32, kind="ExternalInput").ap()
    out = nc.dram_tensor("out", [NH, F], FP32, kind="ExternalOutput").ap()

    with TileContext(nc) as tc:
        with (
            tc.tile_pool(name="const", bufs=1) as cpool,
            tc.tile_pool(name="xs", bufs=3) as xpool,
            tc.tile_pool(name="pr", bufs=2) as prpool,
            tc.tile_pool(name="sm", bufs=2) as smpool,
            tc.tile_pool(name="ac", bufs=1) as acpool,
            tc.tile_pool(name="ps", bufs=2, space="PSUM") as pspool,
        ):
            a_sb = cpool.tile([128, N], FP32)
            nc.sync.dma_start(out=a_sb[:, :], in_=a[:, :])
            d_sb = cpool.tile([128, F], FP32)
            nc.sync.dma_start(out=d_sb[:, :], in_=d[:, :])
            eye_sb = cpool.tile([128, 128], FP32)
            nc.sync.dma_start(out=eye_sb[:, :], in_=eye[:, :])

            acc = acpool.tile([128, F], FP32)

            j0 = 0
            for c, CJ in enumerate(CJS):
                xts = []
                for k in range(3):
                    xt = xpool.tile(
                        [128, CJ * F], FP32, name=f"xt{k}", tag=f"xt{k}"
                    )
                    src = bass.AP(
                        xk.tensor,
                        k * NH * N * F + j0 * F,
                        [[N * F, 128], [1, CJ * F]],
                    )
                    nc.sync.dma_start(out=xt[:, :], in_=src)
                    xts.append(xt)
                if c in PE_CHUNKS:
                    # hop sum on the TensorEngine: identity-stationary
                    # matmuls accumulate x1+x2+x3 into PSUM; PE reads SBUF
                    # through its own ports, so DVE is not slowed
                    ps = pspool.tile([128, CJ * F], FP32, name="ps", tag="ps")
                    for s in range((CJ * F) // 512):
                        sl = slice(s * 512, (s + 1) * 512)
                        for k in range(3):
                            nc.tensor.matmul(
                                ps[:, sl],
                                eye_sb[:, :],
                                xts[k][:, sl],
                                start=(k == 0),
                                stop=(k == 2),
                            )
                    xs = ps
                else:
                    # hop sum on DVE (in place)
                    nc.vector.tensor_add(xts[1][:, :], xts[1][:, :], xts[2][:, :])
                    nc.vector.tensor_add(xts[0][:, :], xts[0][:, :], xts[1][:, :])
                    xs = xts[0]
                xs_step = xs.ap[0][0]

                # prod[i, j*F+f] = xs[i, j*F+f] * a_sb[i, c*CJ+j]
                prod = prpool.tile([128, CJ * F], FP32, name="prod", tag="prod")
                pr_step = prod.ap[0][0]
                a_step = a_sb.ap[0][0]
                in0 = bass.AP(xs.tensor, 0, [[xs_step, 128], [F, CJ], [1, F]])
                in1 = bass.AP(
                    a_sb.tensor, j0, [[a_step, 128], [1, CJ], [0, F]]
                )
                j0 += CJ
                po = bass.AP(prod.tensor, 0, [[pr_step, 128], [F, CJ], [1, F]])
                nc.vector.tensor_mul(po, in0, in1)

                # partial[i, f] = sum_j prod[i, j*F+f]  (reduce innermost=j)
                partial = smpool.tile([128, F], FP32, name="partial", tag="partial")
                pin = bass.AP(prod.tensor, 0, [[pr_step, 128], [1, F], [F, CJ]])
                nc.vector.reduce_sum(
                    partial[:, :], pin, axis=mybir.AxisListType.X
                )

                if c == 0:
                    nc.vector.tensor_add(acc[:, :], d_sb[:, :], partial[:, :])
                else:
                    nc.vector.tensor_add(acc[:, :], acc[:, :], partial[:, :])

            nc.sync.dma_start(out=out[:, :], in_=acc[:, :])

    nc.compile()
    _CACHE["nc"] = nc
    return nc


def _make_in_maps(A, X):
    idx = np.arange(NH)
    in_maps = []
    for c in range(8):
        b, h = c // 2, c % 2
        lo = h * NH
        xk = np.ascontiguousarray(X[b, 1:4, lo : lo + NH])
        av = np.ascontiguousarray(A[b, lo : lo + NH, :])
        dv = np.ascontiguousarray(X[b, 0, lo + idx, lo + idx, :])
        in_maps.append(
            {"xk": xk, "a": av, "d": dv, "eye": np.eye(128, dtype=np.float32)}
        )
    return in_maps


def run(A, X, trace=False, **kw):
    nc = _build_nc()
    in_maps = _make_in_maps(A, X)
    res = run_bass_kernel_spmd(
        nc, in_maps, core_ids=list(range(8)), trace=trace, **kw
    )
    out = np.empty((BATCH, N, F), dtype=np.float32)
    for c in range(8):
        b, h = c // 2, c % 2
        out[b, h * NH : (h + 1) * NH] = res.results[c]["out"]
    return out, res


def kernel(A, X):
    A = np.asarray(A, dtype=np.float32)
    X = np.asarray(X, dtype=np.float32)
    out, _ = run(A, X, trace=False)
    return out
